# revision 17
# baseline (speedup 1.0000x reference)
"""Trainium2 Bass kernel for nn_ExactSpectralHead (sparse resonance attention).

Reference (per batch b):  q,k,v = x@W{q,k,v}.T;  s = qk^T/sqrt(C) + bias;
  p = softmax(where(allowed, s, -inf));  out = p@v.

Strategy (8 cores, one batch element per core):
  - Positions whose `allowed` row AND column are diagonal-only ("empty"
    positions: no shared basis-prime factor with anyone) attend only to
    themselves, and nobody attends to them => out = v for those rows.
    Permute the sequence so the 1730 non-empty positions (plus 62 empty
    pads) occupy slots 0..1791; the trailing 256 empty rows are served by
    a direct v copy. The attention problem shrinks to 1792x1792.
  - EB = exp(bias)*allowed folded host-side (bf16, values are small ints,
    exact). p_raw = exp(qk/sqrt(C)) * EB; normalization deferred to a
    row-sum after the PV matmul.
  - Tiles: key chunks of 128 (PSUM partitions) x query blocks of 448.
    Per (chunk, block) tile the host computes the active column range
    from EB; fully-zero tiles are skipped; chunk pairs share a union
    range so exp/mul/matmuls all run narrowed.
  - Q/K projections in fp8 with DoubleRow perf mode (2 contraction rows
    per partition -> 2x rate; q/k only feed the exponent, accuracy ok).
    V stays bf16, computed as VT = Wv.T^T @ xT then PE-transposed into
    [tk,H] chunks for the PV stationary operand.
  - All post-softmax elementwise work is 2-byte dtype in SBUF so the DVE
    runs in its 4x/2x fast modes: exp (ACT) -> ptb fp16, EB-mul (DVE 4x)
    -> pt fp16, chunk-pair adds (DVE 4x) -> pair sums; rowsum = small
    bf16 ones-matmuls over pair sums; 1/rowsum via reciprocal_approx_fast.
  - PE stream kept dense (pstate ramp) by interleaving projection and
    transpose matmuls between score matmuls; OT(PV) matmuls trail the
    score matmuls by >=2 pairs so the ACT/DVE chase never stalls the PE.
"""

import sys

sys.path.insert(0, "/opt/trn_rl_repo")

import numpy as np
import ml_dtypes

import concourse.bass as bass
import concourse.tile as tile
import concourse.mybir as mybir

# ----------------------------------------------------------------------------
# Workaround for walrus codegen "Too many sync wait commands" on the
# TileContext tail Drain: split the global-clock sem waits across multiple SP
# NOP instructions instead of attaching them all to the single Drain.
from concourse.vector_clock import ScopedClock, VectorClock


def _split_drain_and_barrier(self, tick_clock, wait_clock):
    import concourse.mybir as _mybir

    nc = self.nc
    gc = tick_clock.global_clock
    n = len(gc)
    for p in range(n):
        t = gc[p]
        if t > 0:
            nop = nc.sync.nop(nofuse=True, hint=f"drain_wait_{p}")
            vc = VectorClock([t if i == p else 0 for i in range(n)])
            wait_clock.add_sem_waits(nop.ins, ScopedClock({None: vc}))

    tail_sem = nc.alloc_semaphore("tile_tail_sem")
    n_signals = 0
    for etype, eng in nc.engines.items():
        if etype == _mybir.EngineType.Pool:
            continue
        eng.drain(fusable=False)
        eng.sem_inc(tail_sem, 1)
        n_signals += 1
    nc.gpsimd.wait_ge(tail_sem, n_signals)
    assert self.sems is not None
    popped = nc._tile_sem_poison_stack.pop()
    assert popped is self._sem_poison
    nc.clear_and_free_semaphores(list(self.sems.allocated().values()))
    nc.gpsimd.sem_clear(range(tail_sem.num, tail_sem.num + 1))


tile.TileContext._drain_and_barrier = _split_drain_and_barrier
# ----------------------------------------------------------------------------


def _split_excess_waits(nc, max_waits=1):
    """Walrus codegen supports only one sem-wait per instruction; hoist excess
    waits onto preceding same-engine NOPs, and replace the slow EventSemaphore
    ops with NoOps carrying the same sync_info."""
    for f in nc.m.functions:
        for bb in f.blocks:
            new = []
            changed = False
            for inst in bb.instructions:
                if isinstance(inst, mybir.InstEventSemaphore):
                    si = inst.sync_info
                    changed = True
                    w = list(si.on_wait) if si else []
                    u = list(si.on_update) if si else []
                    if w:
                        new.append(
                            mybir.InstNoOp(
                                name=f"{inst.name}-wait",
                                engine=inst.engine,
                                bass_nofuse=True,
                                sync_info=mybir.SyncInfo(on_wait=w, on_update=[]),
                            )
                        )
                    new.append(
                        mybir.InstNoOp(
                            name=inst.name,
                            engine=inst.engine,
                            bass_nofuse=True,
                            sync_info=mybir.SyncInfo(on_wait=[], on_update=u),
                        )
                    )
                    continue
                si = inst.sync_info
                waits = list(si.on_wait) if si is not None else []
                if len(waits) > max_waits:
                    changed = True
                    excess, keep = waits[:-max_waits], waits[-max_waits:]
                    for k, w in enumerate(excess):
                        new.append(
                            mybir.InstNoOp(
                                name=f"{inst.name}-w{k}",
                                engine=inst.engine,
                                bass_nofuse=True,
                                sync_info=mybir.SyncInfo(on_wait=[w], on_update=[]),
                            )
                        )
                    inst.sync_info = mybir.SyncInfo(
                        on_wait=keep, on_update=list(si.on_update)
                    )
                new.append(inst)
            if changed:
                bb.instructions = new


B, T, C, H = 8, 2048, 1024, 128
NCORES = 8
SCALE = float(C) ** -0.5
P = 128
TQ = 448                   # query block width
NJ = 4                     # query blocks (4*448 = 1792 active positions)
NACT = NJ * TQ             # 1792
NKC = NACT // P            # 14 key chunks
NTAIL = T - NACT           # 256 empty-tail positions served by v-copy
BF16 = mybir.dt.bfloat16
FP16 = mybir.dt.float16
FP8 = mybir.dt.float8e4
F32 = mybir.dt.float32
DR = mybir.MatmulPerfMode.DoubleRow

_nc_cache = None
_sched_cache = None


def _schedule(allowed):
    """Permutation + per-block pair schedule, derived from `allowed`."""
    allowed = np.asarray(allowed, dtype=bool)
    row1 = allowed.sum(1) == 1
    col1 = allowed.sum(0) == 1
    empty = row1 & col1
    nonempty_idx = np.where(~empty)[0]
    empty_idx = np.where(empty)[0]
    npad = NACT - len(nonempty_idx)
    assert npad >= 0
    perm = np.concatenate([nonempty_idx, empty_idx[:npad]])
    full_perm = np.concatenate([perm, empty_idx[npad:]])
    nz = allowed[np.ix_(perm, perm)]  # [q, k]

    blocks = []  # per block: list of (i0, i1|None, lo, hi)
    for j in range(NJ):
        bq = nz[j * TQ:(j + 1) * TQ]
        act = []
        for i in range(NKC):
            colnz = bq[:, i * P:(i + 1) * P].any(axis=1)
            if not colnz.any():
                continue
            qlo = int(np.argmax(colnz))
            qhi = TQ - int(np.argmax(colnz[::-1]))
            act.append((i, qlo, qhi))
        idxs = [a[0] for a in act]
        assert idxs == list(range(len(idxs))), f"non-contiguous actives {idxs}"
        pairs = []
        k = 0
        while k < len(act):
            if k + 1 < len(act):
                lo = min(act[k][1], act[k + 1][1])
                hi = max(act[k][2], act[k + 1][2])
                pairs.append([act[k][0], act[k + 1][0], lo, hi])
                k += 2
            else:
                pairs.append([act[k][0], None, act[k][1], act[k][2]])
                k += 1
        pairs[0][2], pairs[0][3] = 0, TQ  # first pair covers full width
        blocks.append([tuple(p) for p in pairs])

    # eb packing offsets (elements per partition), consumption order
    offs = []
    off = 0
    for j in range(NJ):
        boffs = []
        for (i0, i1, lo, hi) in blocks[j]:
            w = hi - lo
            n = 2 if i1 is not None else 1
            boffs.append(off)
            off += n * w
        offs.append(boffs)
    return full_perm, blocks, offs, off


def _build_nc(blocks, offs, ebw):
    nc = bass.Bass()
    # host-packed DRAM inputs
    x16t = nc.declare_dram_parameter("x16t", [4, P, 8, 512], BF16, isOutput=False)
    x8t = nc.declare_dram_parameter("x8t", [4, P, 8, TQ], FP8, isOutput=False)
    wqk8 = nc.declare_dram_parameter("wqk8", [P, 2, 4, 2, H], FP8, isOutput=False)
    # wv chunks + identity packed in one bf16 buffer
    wvi = nc.declare_dram_parameter("wvi", [P, 8 * H + P], BF16, isOutput=False)
    ebt = nc.declare_dram_parameter("ebt", [P, ebw], BF16, isOutput=False)
    outt = nc.declare_dram_parameter("outt", [NJ, H, TQ], FP16, isOutput=True)
    outr = nc.declare_dram_parameter("outr", [1, NJ, TQ], FP16, isOutput=True)
    outv = nc.declare_dram_parameter("outv", [P, NTAIL], BF16, isOutput=True)

    with tile.TileContext(nc) as tc:
        with (
            tc.tile_pool(name="const", bufs=1) as const,
            tc.tile_pool(name="qkv_psum", bufs=1, space="PSUM") as qkv_psum,
            tc.tile_pool(name="st_psum", bufs=2, space="PSUM") as st_psum,
            tc.tile_pool(name="ot_psum", bufs=2, space="PSUM") as ot_psum,
            tc.tile_pool(name="rs_psum", bufs=1, space="PSUM") as rs_psum,
            tc.tile_pool(name="ptb", bufs=3) as ptb_pool,
            tc.tile_pool(name="pt", bufs=8) as pt_pool,
            tc.tile_pool(name="ps", bufs=9) as ps_pool,
            tc.tile_pool(name="outs", bufs=2) as out_pool,
        ):
            x_sb = const.tile([P, 4, 8, 512], BF16, tag="x", name="x_sb")
            x8_sb = const.tile([P, 4, 8, TQ], FP8, tag="x8", name="x8_sb")
            w8_sb = const.tile([P, 2, 4, 2, H], FP8, tag="w8", name="w8_sb")
            wvi_sb = const.tile([P, 8 * H + P], BF16, tag="wvi", name="wvi_sb")
            rsn_sb = const.tile([1, NJ, TQ], FP16, tag="rsn", name="rsn_sb")
            eb_sb = const.tile([P, ebw], BF16, tag="eb", name="eb_sb")
            QT_sb = const.tile([P, NACT], BF16, tag="QT", name="QT_sb")
            KT_sb = const.tile([P, NACT], BF16, tag="KT", name="KT_sb")
            VT_sb = const.tile([P, T], BF16, tag="VT", name="VT_sb")
            v_sb = const.tile([P, NKC, H], BF16, tag="v", name="v_sb")
            ones_sb = const.tile([P, P], BF16, tag="ones", name="ones_sb")
            nc.vector.memset(ones_sb[:], 1.0)
            wv_sb = wvi_sb[:, 0:8 * H].rearrange("p (c h) -> p c h", c=8)
            id_sb = wvi_sb[:, 8 * H:8 * H + P]

            # ---- t0 DMA batch: critical-path pieces first on hardware queues
            with tc.high_priority():
                nc.scalar.dma_start(w8_sb[:], wqk8[:])
                nc.sync.dma_start(x8_sb[:, 0], x8t[0])
                nc.sync.dma_start(x8_sb[:, 1], x8t[1])
                nc.sync.dma_start(eb_sb[:, offs[0][0]:offs[1][0]],
                                  ebt[:, offs[0][0]:offs[1][0]])
                nc.sync.dma_start(x8_sb[:, 2], x8t[2])
                nc.sync.dma_start(x8_sb[:, 3], x8t[3])
                nc.gpsimd.dma_start(x_sb[:, 0, 0:4], x16t[0][:, 0:4])
                nc.gpsimd.dma_start(x_sb[:, 0, 4:8], x16t[0][:, 4:8])
                nc.gpsimd.dma_start(wvi_sb[:], wvi[:])
                nc.sync.dma_start(x_sb[:, 1], x16t[1])
                nc.sync.dma_start(eb_sb[:, offs[1][0]:offs[2][0]],
                                  ebt[:, offs[1][0]:offs[2][0]])
                nc.gpsimd.dma_start(x_sb[:, 2], x16t[2])
                nc.gpsimd.dma_start(eb_sb[:, offs[2][0]:offs[3][0]],
                                    ebt[:, offs[2][0]:offs[3][0]])
                nc.gpsimd.dma_start(x_sb[:, 3], x16t[3])
                nc.gpsimd.dma_start(eb_sb[:, offs[3][0]:ebw], ebt[:, offs[3][0]:ebw])

            # ---------- emission helpers ----------
            def qk(j):
                """Q/K projections for block j (fp8 DoubleRow, 4 c-pairs)."""
                for wsel, dst in ((0, QT_sb), (1, KT_sb)):
                    ps = qkv_psum.tile([P, 512], F32, tag="qkvps", name="qkvps")
                    for pair in range(4):
                        nc.tensor.matmul(
                            ps[:, :TQ],
                            lhsT=w8_sb[:, wsel, pair],
                            rhs=x8_sb[:, j, 2 * pair:2 * pair + 2, :],
                            start=(pair == 0),
                            stop=(pair == 3),
                            perf_mode=DR,
                        )
                    nc.vector.tensor_copy(dst[:, j * TQ:(j + 1) * TQ], ps[:, :TQ])

            def vt(vb):
                """V^T projection for 512-col block vb (bf16)."""
                ps = qkv_psum.tile([P, 512], F32, tag="qkvps", name="qkvps")
                for c in range(8):
                    nc.tensor.matmul(
                        ps[:],
                        lhsT=wv_sb[:, c, :],
                        rhs=x_sb[:, vb, c, :],
                        start=(c == 0),
                        stop=(c == 7),
                    )
                nc.vector.tensor_copy(VT_sb[:, vb * 512:(vb + 1) * 512], ps[:])
                if vb == 3:
                    nc.sync.dma_start(outv[:], VT_sb[:, NACT:T])

            def tr(m):
                """PE-transpose VT chunk m into v_sb[:, m, :]."""
                pst = qkv_psum.tile([P, 1024], BF16, tag="qkvps", name="qkvps")
                nc.tensor.transpose(pst[:, :P], VT_sb[:, m * P:(m + 1) * P], id_sb)
                nc.vector.tensor_copy(v_sb[:, m, :], pst[:, :P])

            class Blk:
                pass

            def blk_start(j):
                b = Blk()
                b.j = j
                b.pairs = blocks[j]
                b.ot = ot_psum.tile([P, 512], F32, tag="ot", name="ot")
                b.rs = rs_psum.tile([P, 512], F32, tag="rs", name="rs")
                b.pts = []
                b.qsums = []
                b.nt = sum(2 if i1 is not None else 1 for (i0, i1, _, _) in b.pairs)
                b.ti = 0
                return b

            def st(b, p):
                """Score matmuls + exp + EB-mul + pair-sum for pair p."""
                i0, i1, lo, hi = b.pairs[p]
                n = 2 if i1 is not None else 1
                st2 = st_psum.tile([P, 2, 512], F32, tag="st", name="st2")
                for k, i in enumerate((i0, i1)[:n]):
                    nc.tensor.matmul(
                        st2[:, k, lo:hi],
                        lhsT=KT_sb[:, i * P:(i + 1) * P],
                        rhs=QT_sb[:, b.j * TQ + lo:b.j * TQ + hi],
                        start=True,
                        stop=True,
                    )
                ptb = ptb_pool.tile([P, 2, TQ], FP16, tag="ptb", name="ptb")
                nc.scalar.activation(
                    ptb[:, :n, lo:hi], st2[:, :n, lo:hi],
                    mybir.ActivationFunctionType.Exp, scale=SCALE,
                )
                pt = pt_pool.tile([P, 2, TQ], FP16, tag="pt", name="pt")
                off = offs[b.j][p]
                w = hi - lo
                for k in range(n):
                    nc.vector.tensor_mul(
                        pt[:, k, lo:hi],
                        ptb[:, k, lo:hi],
                        eb_sb[:, off + k * w:off + (k + 1) * w],
                    )
                b.pts.append(pt)
                if p % 2 == 0:
                    # new quad accumulator (pair p's range contains pair p+1's)
                    qs = ps_pool.tile([P, TQ], FP16, tag="psum", name="psum")
                    if n == 2:
                        nc.vector.tensor_add(qs[:, lo:hi], pt[:, 0, lo:hi], pt[:, 1, lo:hi])
                    else:
                        nc.vector.tensor_copy(qs[:, lo:hi], pt[:, 0, lo:hi])
                    b.qsums.append((qs, lo, hi))
                else:
                    qs, qlo, qhi = b.qsums[-1]
                    assert qlo <= lo and hi <= qhi, "pair ranges not nested"
                    for k in range(n):
                        nc.vector.tensor_add(qs[:, lo:hi], qs[:, lo:hi], pt[:, k, lo:hi])

            def ot(b, p):
                i0, i1, lo, hi = b.pairs[p]
                n = 2 if i1 is not None else 1
                for k, i in enumerate((i0, i1)[:n]):
                    nc.tensor.matmul(
                        b.ot[:, lo:hi],
                        lhsT=v_sb[:, i, :],
                        rhs=b.pts[p][:, k, lo:hi],
                        start=(b.ti == 0),
                        stop=(b.ti == b.nt - 1),
                        skip_group_check=True,
                    )
                    b.ti += 1

            def rs_all(b):
                nq = len(b.qsums)
                for q, (qs, lo, hi) in enumerate(b.qsums):
                    nc.tensor.matmul(
                        b.rs[:, lo:hi],
                        lhsT=ones_sb[:],
                        rhs=qs[:, lo:hi],
                        start=(q == 0),
                        stop=(q == nq - 1),
                        skip_group_check=True,
                    )

            def epi(b):
                otn = out_pool.tile([P, TQ], FP16, tag="otn", name="otn")
                nc.vector.tensor_copy(otn[:], b.ot[:, :TQ])
                nc.vector.tensor_copy(rsn_sb[:, b.j, :], b.rs[0:1, :TQ])
                nc.sync.dma_start(outt[b.j], otn[:])
                if b.j == NJ - 1:
                    nc.sync.dma_start(outr[:], rsn_sb[:])

            # ---------- global emission ----------
            # OT/RS for block j-1 are interleaved into block j's ST phase so
            # the PE always has ready work while the exp/mul chase runs.
            qk(0)
            qk(1)
            qk(2)
            qk(3)

            bs = [blk_start(j) for j in range(NJ)]

            def fillers_for(j):
                if j == 0:
                    return [lambda: vt(0),
                            lambda: tr(0), lambda: tr(1), lambda: tr(2),
                            lambda: tr(3), lambda: vt(1)]
                if j == 1:
                    return [lambda: tr(4), lambda: tr(5), lambda: tr(6),
                            lambda: tr(7), lambda: vt(2)]
                if j == 2:
                    return [lambda: tr(8), lambda: tr(9), lambda: tr(10),
                            lambda: tr(11), lambda: vt(3)]
                return [lambda: tr(12), lambda: tr(13)]

            for j in range(NJ):
                b = bs[j]
                prev = bs[j - 1] if j > 0 else None
                fill = fillers_for(j)
                lag = list(range(len(prev.pairs))) if prev is not None else []
                li = 0
                fi = 0
                own = 0
                for p in range(len(b.pairs)):
                    # pad with one lagged OT pair + one filler per ST pair
                    if li < len(lag):
                        ot(prev, lag[li])
                        li += 1
                    if fi < len(fill):
                        fill[fi]()
                        fi += 1
                    st(b, p)
                    if j == NJ - 1 and p >= 4:
                        # last block: start its own PV early to shrink the tail
                        if li < len(lag):
                            ot(prev, li)
                            li += 1
                        ot(b, own)
                        own += 1
                while fi < len(fill):
                    fill[fi]()
                    fi += 1
                while li < len(lag):
                    ot(prev, li)
                    li += 1
                if prev is not None:
                    rs_all(prev)
                    epi(prev)
                if j == NJ - 1:
                    while own < len(b.pairs):
                        ot(b, own)
                        own += 1
                    rs_all(b)
                    epi(b)

    _split_excess_waits(nc)
    return nc


def kernel(x, Wq, Wk, Wv, resonance_bias, allowed):
    global _nc_cache, _sched_cache
    x = np.asarray(x, dtype=np.float32)
    Wq = np.asarray(Wq, dtype=np.float32)
    Wk = np.asarray(Wk, dtype=np.float32)
    Wv = np.asarray(Wv, dtype=np.float32)
    resonance_bias = np.asarray(resonance_bias, dtype=np.float32)
    allowed = np.asarray(allowed)

    bf16 = ml_dtypes.bfloat16
    fp8 = ml_dtypes.float8_e4m3

    if _sched_cache is None:
        _sched_cache = _schedule(allowed)
    full_perm, blocks, offs, ebw = _sched_cache
    if _nc_cache is None:
        _nc_cache = _build_nc(blocks, offs, ebw)
    nc = _nc_cache

    # ---- host packing ----
    EB = np.exp(resonance_bias) * allowed
    EBp = EB[np.ix_(full_perm[:NACT], full_perm[:NACT])]  # [q, k]
    ebT = np.ascontiguousarray(EBp.T)                      # [k, q]
    eb_pack = np.empty((P, ebw), dtype=bf16)
    for j in range(NJ):
        for p, (i0, i1, lo, hi) in enumerate(blocks[j]):
            off = offs[j][p]
            w = hi - lo
            qs = slice(j * TQ + lo, j * TQ + hi)
            eb_pack[:, off:off + w] = ebT[i0 * P:(i0 + 1) * P, qs].astype(bf16)
            if i1 is not None:
                eb_pack[:, off + w:off + 2 * w] = ebT[i1 * P:(i1 + 1) * P, qs].astype(bf16)

    wqk8 = np.ascontiguousarray(
        np.stack(
            [w.reshape(4, 2, P, H).transpose(2, 0, 1, 3)
             for w in (np.ascontiguousarray(Wq.T), np.ascontiguousarray(Wk.T))],
            axis=1,
        ).astype(fp8)
    )
    wvi = np.concatenate(
        [
            Wv.T.reshape(8, P, H).transpose(1, 0, 2).reshape(P, 8 * H),
            np.eye(P, dtype=np.float32),
        ],
        axis=1,
    ).astype(bf16)

    in_maps = []
    for b in range(NCORES):
        xT = x[b].T[:, full_perm]                      # [C, T] permuted cols
        xr = xT.reshape(8, P, T)                       # [c, p, t]
        x16t = np.ascontiguousarray(
            xr.reshape(8, P, 4, 512).transpose(2, 1, 0, 3)
        ).astype(bf16)                                 # [4, P, 8, 512]
        x8t = np.ascontiguousarray(
            xr[:, :, :NACT].reshape(8, P, 4, TQ).transpose(2, 1, 0, 3)
        ).astype(fp8)                                  # [4, P, 8, 448]
        in_maps.append(
            {
                "x16t": x16t,
                "x8t": x8t,
                "wqk8": wqk8,
                "wvi": wvi,
                "ebt": eb_pack,
            }
        )

    from concourse import bass2jax

    try:
        results = bass2jax.run_bass_via_pjrt(nc, in_maps, n_cores=NCORES)
    except Exception:
        import time as _time

        _time.sleep(2.0)
        results = bass2jax.run_bass_via_pjrt(nc, in_maps, n_cores=NCORES)

    out = np.empty((B, T, H), dtype=np.float32)
    inv = np.argsort(full_perm)
    for b in range(NCORES):
        outt = np.asarray(results[b]["outt"]).astype(np.float32)  # [NJ, H, TQ]
        outr = np.asarray(results[b]["outr"]).astype(np.float32)  # [1, NJ, TQ]
        outt /= outr.reshape(NJ, 1, TQ)
        oattn = outt.transpose(0, 2, 1).reshape(NACT, H)
        outv = np.asarray(results[b]["outv"]).astype(np.float32)  # [H, NTAIL]
        full = np.concatenate([oattn, outv.T], axis=0)  # [T, H] in perm order
        out[b] = full[inv]
    return out


# revision 19
# speedup vs baseline: 1.1569x; 1.1569x over previous
"""Trainium2 Bass kernel for nn_ExactSpectralHead (sparse resonance attention).

Reference (per batch b):  q,k,v = x@W{q,k,v}.T;  s = qk^T/sqrt(C) + bias;
  p = softmax(where(allowed, s, -inf));  out = p@v.

Strategy (8 cores, one batch element per core):
  - Positions whose `allowed` row AND column are diagonal-only ("empty"
    positions: no shared basis-prime factor with anyone) attend only to
    themselves, and nobody attends to them => out = v for those rows.
    Permute the sequence so the 1730 non-empty positions (plus 62 empty
    pads) occupy slots 0..1791; the trailing 256 empty rows are served by
    a direct v copy. The attention problem shrinks to 1792x1792.
  - EB = exp(bias)*allowed folded host-side (bf16, values are small ints,
    exact). p_raw = exp(qk/sqrt(C)) * EB; normalization deferred to a
    row-sum after the PV matmul.
  - Tiles: key chunks of 128 (PSUM partitions) x query blocks of 448.
    Per (chunk, block) tile the host computes the active column range
    from EB; fully-zero tiles are skipped; chunk pairs share a union
    range so exp/mul/matmuls all run narrowed.
  - Q/K projections in fp8 with DoubleRow perf mode (2 contraction rows
    per partition -> 2x rate; q/k only feed the exponent, accuracy ok).
    V stays bf16, computed as VT = Wv.T^T @ xT then PE-transposed into
    [tk,H] chunks for the PV stationary operand.
  - All post-softmax elementwise work is 2-byte dtype in SBUF so the DVE
    runs in its 4x/2x fast modes: exp (ACT) -> ptb fp16, EB-mul (DVE 4x)
    -> pt fp16, chunk-pair adds (DVE 4x) -> pair sums; rowsum = small
    bf16 ones-matmuls over pair sums; 1/rowsum via reciprocal_approx_fast.
  - PE stream kept dense (pstate ramp) by interleaving projection and
    transpose matmuls between score matmuls; OT(PV) matmuls trail the
    score matmuls by >=2 pairs so the ACT/DVE chase never stalls the PE.
"""

import sys

sys.path.insert(0, "/opt/trn_rl_repo")

import numpy as np
import ml_dtypes

import concourse.bass as bass
import concourse.tile as tile
import concourse.mybir as mybir

# ----------------------------------------------------------------------------
# Workaround for walrus codegen "Too many sync wait commands" on the
# TileContext tail Drain: split the global-clock sem waits across multiple SP
# NOP instructions instead of attaching them all to the single Drain.
from concourse.vector_clock import ScopedClock, VectorClock


def _split_drain_and_barrier(self, tick_clock, wait_clock):
    import concourse.mybir as _mybir

    nc = self.nc
    gc = tick_clock.global_clock
    n = len(gc)
    for p in range(n):
        t = gc[p]
        if t > 0:
            nop = nc.sync.nop(nofuse=True, hint=f"drain_wait_{p}")
            vc = VectorClock([t if i == p else 0 for i in range(n)])
            wait_clock.add_sem_waits(nop.ins, ScopedClock({None: vc}))

    tail_sem = nc.alloc_semaphore("tile_tail_sem")
    n_signals = 0
    for etype, eng in nc.engines.items():
        if etype == _mybir.EngineType.Pool:
            continue
        eng.drain(fusable=False)
        eng.sem_inc(tail_sem, 1)
        n_signals += 1
    nc.gpsimd.wait_ge(tail_sem, n_signals)
    assert self.sems is not None
    popped = nc._tile_sem_poison_stack.pop()
    assert popped is self._sem_poison
    nc.clear_and_free_semaphores(list(self.sems.allocated().values()))
    nc.gpsimd.sem_clear(range(tail_sem.num, tail_sem.num + 1))


tile.TileContext._drain_and_barrier = _split_drain_and_barrier
# ----------------------------------------------------------------------------


def _split_excess_waits(nc, max_waits=1):
    """Walrus codegen supports only one sem-wait per instruction; hoist excess
    waits onto preceding same-engine NOPs, and replace the slow EventSemaphore
    ops with NoOps carrying the same sync_info."""
    for f in nc.m.functions:
        for bb in f.blocks:
            new = []
            changed = False
            for inst in bb.instructions:
                if isinstance(inst, mybir.InstEventSemaphore):
                    si = inst.sync_info
                    changed = True
                    w = list(si.on_wait) if si else []
                    u = list(si.on_update) if si else []
                    if w:
                        new.append(
                            mybir.InstNoOp(
                                name=f"{inst.name}-wait",
                                engine=inst.engine,
                                bass_nofuse=True,
                                sync_info=mybir.SyncInfo(on_wait=w, on_update=[]),
                            )
                        )
                    new.append(
                        mybir.InstNoOp(
                            name=inst.name,
                            engine=inst.engine,
                            bass_nofuse=True,
                            sync_info=mybir.SyncInfo(on_wait=[], on_update=u),
                        )
                    )
                    continue
                si = inst.sync_info
                waits = list(si.on_wait) if si is not None else []
                if len(waits) > max_waits:
                    changed = True
                    excess, keep = waits[:-max_waits], waits[-max_waits:]
                    for k, w in enumerate(excess):
                        new.append(
                            mybir.InstNoOp(
                                name=f"{inst.name}-w{k}",
                                engine=inst.engine,
                                bass_nofuse=True,
                                sync_info=mybir.SyncInfo(on_wait=[w], on_update=[]),
                            )
                        )
                    inst.sync_info = mybir.SyncInfo(
                        on_wait=keep, on_update=list(si.on_update)
                    )
                new.append(inst)
            if changed:
                bb.instructions = new


B, T, C, H = 8, 2048, 1024, 128
NCORES = 8
SCALE = float(C) ** -0.5
P = 128
TQ = 448                   # query block width
NJ = 4                     # query blocks (4*448 = 1792 active positions)
NACT = NJ * TQ             # 1792
NKC = NACT // P            # 14 key chunks
NTAIL = T - NACT           # 256 empty-tail positions served by v-copy
BF16 = mybir.dt.bfloat16
FP16 = mybir.dt.float16
FP8 = mybir.dt.float8e4
F32 = mybir.dt.float32
DR = mybir.MatmulPerfMode.DoubleRow

_nc_cache = None
_sched_cache = None


def _schedule(allowed):
    """Permutation + per-block pair schedule, derived from `allowed`."""
    allowed = np.asarray(allowed, dtype=bool)
    row1 = allowed.sum(1) == 1
    col1 = allowed.sum(0) == 1
    empty = row1 & col1
    nonempty_idx = np.where(~empty)[0]
    empty_idx = np.where(empty)[0]
    npad = NACT - len(nonempty_idx)
    assert npad >= 0
    perm = np.concatenate([nonempty_idx, empty_idx[:npad]])
    full_perm = np.concatenate([perm, empty_idx[npad:]])
    nz = allowed[np.ix_(perm, perm)]  # [q, k]

    blocks = []  # per block: list of (i0, i1|None, lo, hi)
    for j in range(NJ):
        bq = nz[j * TQ:(j + 1) * TQ]
        act = []
        for i in range(NKC):
            colnz = bq[:, i * P:(i + 1) * P].any(axis=1)
            if not colnz.any():
                continue
            qlo = int(np.argmax(colnz))
            qhi = TQ - int(np.argmax(colnz[::-1]))
            act.append((i, qlo, qhi))
        idxs = [a[0] for a in act]
        assert idxs == list(range(len(idxs))), f"non-contiguous actives {idxs}"
        pairs = []
        k = 0
        while k < len(act):
            if k + 1 < len(act):
                lo = min(act[k][1], act[k + 1][1])
                hi = max(act[k][2], act[k + 1][2])
                pairs.append([act[k][0], act[k + 1][0], lo, hi])
                k += 2
            else:
                pairs.append([act[k][0], None, act[k][1], act[k][2]])
                k += 1
        pairs[0][2], pairs[0][3] = 0, TQ  # first pair covers full width
        blocks.append([tuple(p) for p in pairs])

    # eb packing offsets (elements per partition), consumption order
    offs = []
    off = 0
    for j in range(NJ):
        boffs = []
        for (i0, i1, lo, hi) in blocks[j]:
            w = hi - lo
            n = 2 if i1 is not None else 1
            boffs.append(off)
            off += n * w
        offs.append(boffs)
    return full_perm, blocks, offs, off


def _build_nc(blocks, offs, ebw):
    nc = bass.Bass()
    # host-packed DRAM inputs
    x16t = nc.declare_dram_parameter("x16t", [4, P, 8, 512], BF16, isOutput=False)
    x8t = nc.declare_dram_parameter("x8t", [4, P, 8, TQ], FP8, isOutput=False)
    wqk8 = nc.declare_dram_parameter("wqk8", [P, 2, 4, 2, H], FP8, isOutput=False)
    # wv chunks + identity packed in one bf16 buffer
    wvi = nc.declare_dram_parameter("wvi", [P, 8 * H + P], BF16, isOutput=False)
    ebt = nc.declare_dram_parameter("ebt", [P, ebw], BF16, isOutput=False)
    outt = nc.declare_dram_parameter("outt", [NJ, H, TQ], FP16, isOutput=True)
    outr = nc.declare_dram_parameter("outr", [1, NJ, TQ], FP16, isOutput=True)
    outv = nc.declare_dram_parameter("outv", [P, NTAIL], BF16, isOutput=True)

    with tile.TileContext(nc) as tc:
        with (
            tc.tile_pool(name="const", bufs=1) as const,
            tc.tile_pool(name="qkv_psum", bufs=1, space="PSUM") as qkv_psum,
            tc.tile_pool(name="st_psum", bufs=2, space="PSUM") as st_psum,
            tc.tile_pool(name="ot_psum", bufs=2, space="PSUM") as ot_psum,
            tc.tile_pool(name="rs_psum", bufs=1, space="PSUM") as rs_psum,
            tc.tile_pool(name="ptb", bufs=3) as ptb_pool,
            tc.tile_pool(name="pt", bufs=8) as pt_pool,
            tc.tile_pool(name="ps", bufs=9) as ps_pool,
            tc.tile_pool(name="outs", bufs=2) as out_pool,
        ):
            x_sb = const.tile([P, 4, 8, 512], BF16, tag="x", name="x_sb")
            x8_sb = const.tile([P, 4, 8, TQ], FP8, tag="x8", name="x8_sb")
            w8_sb = const.tile([P, 2, 4, 2, H], FP8, tag="w8", name="w8_sb")
            wvi_sb = const.tile([P, 8 * H + P], BF16, tag="wvi", name="wvi_sb")
            rsn_sb = const.tile([1, NJ, TQ], FP16, tag="rsn", name="rsn_sb")
            eb_sb = const.tile([P, ebw], BF16, tag="eb", name="eb_sb")
            QT_sb = const.tile([P, NACT], BF16, tag="QT", name="QT_sb")
            KT_sb = const.tile([P, NACT], BF16, tag="KT", name="KT_sb")
            VT_sb = const.tile([P, T], BF16, tag="VT", name="VT_sb")
            v_sb = const.tile([P, NKC, H], BF16, tag="v", name="v_sb")
            ones_sb = const.tile([P, P], BF16, tag="ones", name="ones_sb")
            nc.vector.memset(ones_sb[:], 1.0)
            wv_sb = wvi_sb[:, 0:8 * H].rearrange("p (c h) -> p c h", c=8)
            id_sb = wvi_sb[:, 8 * H:8 * H + P]

            # ---- t0 DMA batch: critical-path pieces first on hardware queues
            with tc.high_priority():
                nc.scalar.dma_start(w8_sb[:], wqk8[:])
                nc.sync.dma_start(x8_sb[:, 0], x8t[0])
                nc.sync.dma_start(x8_sb[:, 1], x8t[1])
                nc.sync.dma_start(x8_sb[:, 2], x8t[2])
                nc.sync.dma_start(x8_sb[:, 3], x8t[3])
                nc.sync.dma_start(eb_sb[:, offs[0][0]:offs[1][0]],
                                  ebt[:, offs[0][0]:offs[1][0]])
                nc.gpsimd.dma_start(x_sb[:, 0, 0:4], x16t[0][:, 0:4])
                nc.gpsimd.dma_start(x_sb[:, 0, 4:8], x16t[0][:, 4:8])
                nc.gpsimd.dma_start(wvi_sb[:], wvi[:])
                nc.gpsimd.dma_start(x_sb[:, 1], x16t[1])
                nc.gpsimd.dma_start(eb_sb[:, offs[1][0]:offs[2][0]],
                                    ebt[:, offs[1][0]:offs[2][0]])
                nc.gpsimd.dma_start(x_sb[:, 2], x16t[2])
                nc.gpsimd.dma_start(eb_sb[:, offs[2][0]:offs[3][0]],
                                    ebt[:, offs[2][0]:offs[3][0]])
                nc.gpsimd.dma_start(x_sb[:, 3], x16t[3])
                nc.gpsimd.dma_start(eb_sb[:, offs[3][0]:ebw], ebt[:, offs[3][0]:ebw])

            # ---------- emission helpers ----------
            def qk(j):
                """Q/K projections for block j (fp8 DoubleRow, 4 c-pairs)."""
                for wsel, dst in ((0, QT_sb), (1, KT_sb)):
                    ps = qkv_psum.tile([P, 512], F32, tag="qkvps", name="qkvps")
                    for pair in range(4):
                        nc.tensor.matmul(
                            ps[:, :TQ],
                            lhsT=w8_sb[:, wsel, pair],
                            rhs=x8_sb[:, j, 2 * pair:2 * pair + 2, :],
                            start=(pair == 0),
                            stop=(pair == 3),
                            perf_mode=DR,
                        )
                    nc.vector.tensor_copy(dst[:, j * TQ:(j + 1) * TQ], ps[:, :TQ])

            def vt(vb):
                """V^T projection for 512-col block vb (bf16)."""
                ps = qkv_psum.tile([P, 512], F32, tag="qkvps", name="qkvps")
                for c in range(8):
                    nc.tensor.matmul(
                        ps[:],
                        lhsT=wv_sb[:, c, :],
                        rhs=x_sb[:, vb, c, :],
                        start=(c == 0),
                        stop=(c == 7),
                    )
                nc.vector.tensor_copy(VT_sb[:, vb * 512:(vb + 1) * 512], ps[:])
                if vb == 3:
                    nc.sync.dma_start(outv[:], VT_sb[:, NACT:T])

            def tr(m):
                """PE-transpose VT chunk m into v_sb[:, m, :]."""
                pst = qkv_psum.tile([P, 1024], BF16, tag="qkvps", name="qkvps")
                nc.tensor.transpose(pst[:, :P], VT_sb[:, m * P:(m + 1) * P], id_sb)
                nc.vector.tensor_copy(v_sb[:, m, :], pst[:, :P])

            class Blk:
                pass

            def blk_start(j):
                b = Blk()
                b.j = j
                b.pairs = blocks[j]
                b.ot = ot_psum.tile([P, 512], F32, tag="ot", name="ot")
                b.rs = rs_psum.tile([P, 512], F32, tag="rs", name="rs")
                b.pts = []
                b.qsums = []
                b.nt = sum(2 if i1 is not None else 1 for (i0, i1, _, _) in b.pairs)
                b.ti = 0
                return b

            def st(b, p):
                """Score matmuls + exp + EB-mul + pair-sum for pair p."""
                i0, i1, lo, hi = b.pairs[p]
                n = 2 if i1 is not None else 1
                st2 = st_psum.tile([P, 2, 512], F32, tag="st", name="st2")
                for k, i in enumerate((i0, i1)[:n]):
                    nc.tensor.matmul(
                        st2[:, k, lo:hi],
                        lhsT=KT_sb[:, i * P:(i + 1) * P],
                        rhs=QT_sb[:, b.j * TQ + lo:b.j * TQ + hi],
                        start=True,
                        stop=True,
                    )
                ptb = ptb_pool.tile([P, 2, TQ], FP16, tag="ptb", name="ptb")
                nc.scalar.activation(
                    ptb[:, :n, lo:hi], st2[:, :n, lo:hi],
                    mybir.ActivationFunctionType.Exp, scale=SCALE,
                )
                pt = pt_pool.tile([P, 2, TQ], FP16, tag="pt", name="pt")
                off = offs[b.j][p]
                w = hi - lo
                for k in range(n):
                    nc.vector.tensor_mul(
                        pt[:, k, lo:hi],
                        ptb[:, k, lo:hi],
                        eb_sb[:, off + k * w:off + (k + 1) * w],
                    )
                b.pts.append(pt)
                if p % 2 == 0:
                    # new quad accumulator (pair p's range contains pair p+1's)
                    qs = ps_pool.tile([P, TQ], FP16, tag="psum", name="psum")
                    if n == 2:
                        nc.vector.tensor_add(qs[:, lo:hi], pt[:, 0, lo:hi], pt[:, 1, lo:hi])
                    else:
                        nc.vector.tensor_copy(qs[:, lo:hi], pt[:, 0, lo:hi])
                    b.qsums.append((qs, lo, hi))
                else:
                    qs, qlo, qhi = b.qsums[-1]
                    assert qlo <= lo and hi <= qhi, "pair ranges not nested"
                    for k in range(n):
                        nc.vector.tensor_add(qs[:, lo:hi], qs[:, lo:hi], pt[:, k, lo:hi])

            def ot(b, p):
                i0, i1, lo, hi = b.pairs[p]
                n = 2 if i1 is not None else 1
                for k, i in enumerate((i0, i1)[:n]):
                    nc.tensor.matmul(
                        b.ot[:, lo:hi],
                        lhsT=v_sb[:, i, :],
                        rhs=b.pts[p][:, k, lo:hi],
                        start=(b.ti == 0),
                        stop=(b.ti == b.nt - 1),
                        skip_group_check=True,
                    )
                    b.ti += 1

            def rs_all(b):
                nq = len(b.qsums)
                for q, (qs, lo, hi) in enumerate(b.qsums):
                    nc.tensor.matmul(
                        b.rs[:, lo:hi],
                        lhsT=ones_sb[:],
                        rhs=qs[:, lo:hi],
                        start=(q == 0),
                        stop=(q == nq - 1),
                        skip_group_check=True,
                    )

            def epi(b):
                otn = out_pool.tile([P, TQ], FP16, tag="otn", name="otn")
                nc.vector.tensor_copy(otn[:], b.ot[:, :TQ])
                nc.vector.tensor_copy(rsn_sb[:, b.j, :], b.rs[0:1, :TQ])
                nc.sync.dma_start(outt[b.j], otn[:])
                if b.j == NJ - 1:
                    nc.sync.dma_start(outr[:], rsn_sb[:])

            # ---------- global emission ----------
            # OT/RS for block j-1 are interleaved into block j's ST phase so
            # the PE always has ready work while the exp/mul chase runs.
            qk(0)
            qk(1)
            qk(2)
            qk(3)

            bs = [blk_start(j) for j in range(NJ)]

            def fillers_for(j):
                if j == 0:
                    return [lambda: vt(0),
                            lambda: tr(0), lambda: tr(1), lambda: tr(2),
                            lambda: tr(3), lambda: vt(1)]
                if j == 1:
                    return [lambda: tr(4), lambda: tr(5), lambda: tr(6),
                            lambda: tr(7), lambda: vt(2)]
                if j == 2:
                    return [lambda: tr(8), lambda: tr(9), lambda: tr(10),
                            lambda: tr(11), lambda: vt(3)]
                return [lambda: tr(12), lambda: tr(13)]

            for j in range(NJ):
                b = bs[j]
                prev = bs[j - 1] if j > 0 else None
                fill = fillers_for(j)
                lag = list(range(len(prev.pairs))) if prev is not None else []
                li = 0
                fi = 0
                own = 0
                for p in range(len(b.pairs)):
                    # pad with one lagged OT pair + one filler per ST pair
                    if li < len(lag):
                        ot(prev, lag[li])
                        li += 1
                    if fi < len(fill):
                        fill[fi]()
                        fi += 1
                    st(b, p)
                    if j == NJ - 1 and p >= 4:
                        # last block: start its own PV early to shrink the tail
                        if li < len(lag):
                            ot(prev, li)
                            li += 1
                        ot(b, own)
                        own += 1
                while fi < len(fill):
                    fill[fi]()
                    fi += 1
                while li < len(lag):
                    ot(prev, li)
                    li += 1
                if prev is not None:
                    rs_all(prev)
                    epi(prev)
                if j == NJ - 1:
                    while own < len(b.pairs):
                        ot(b, own)
                        own += 1
                    rs_all(b)
                    epi(b)

    _split_excess_waits(nc)
    return nc


def kernel(x, Wq, Wk, Wv, resonance_bias, allowed):
    global _nc_cache, _sched_cache
    x = np.asarray(x, dtype=np.float32)
    Wq = np.asarray(Wq, dtype=np.float32)
    Wk = np.asarray(Wk, dtype=np.float32)
    Wv = np.asarray(Wv, dtype=np.float32)
    resonance_bias = np.asarray(resonance_bias, dtype=np.float32)
    allowed = np.asarray(allowed)

    bf16 = ml_dtypes.bfloat16
    fp8 = ml_dtypes.float8_e4m3

    if _sched_cache is None:
        _sched_cache = _schedule(allowed)
    full_perm, blocks, offs, ebw = _sched_cache
    if _nc_cache is None:
        _nc_cache = _build_nc(blocks, offs, ebw)
    nc = _nc_cache

    # ---- host packing ----
    EB = np.exp(resonance_bias) * allowed
    EBp = EB[np.ix_(full_perm[:NACT], full_perm[:NACT])]  # [q, k]
    ebT = np.ascontiguousarray(EBp.T)                      # [k, q]
    eb_pack = np.empty((P, ebw), dtype=bf16)
    for j in range(NJ):
        for p, (i0, i1, lo, hi) in enumerate(blocks[j]):
            off = offs[j][p]
            w = hi - lo
            qs = slice(j * TQ + lo, j * TQ + hi)
            eb_pack[:, off:off + w] = ebT[i0 * P:(i0 + 1) * P, qs].astype(bf16)
            if i1 is not None:
                eb_pack[:, off + w:off + 2 * w] = ebT[i1 * P:(i1 + 1) * P, qs].astype(bf16)

    wqk8 = np.ascontiguousarray(
        np.stack(
            [w.reshape(4, 2, P, H).transpose(2, 0, 1, 3)
             for w in (np.ascontiguousarray(Wq.T), np.ascontiguousarray(Wk.T))],
            axis=1,
        ).astype(fp8)
    )
    wvi = np.concatenate(
        [
            Wv.T.reshape(8, P, H).transpose(1, 0, 2).reshape(P, 8 * H),
            np.eye(P, dtype=np.float32),
        ],
        axis=1,
    ).astype(bf16)

    in_maps = []
    for b in range(NCORES):
        xT = x[b].T[:, full_perm]                      # [C, T] permuted cols
        xr = xT.reshape(8, P, T)                       # [c, p, t]
        x16t = np.ascontiguousarray(
            xr.reshape(8, P, 4, 512).transpose(2, 1, 0, 3)
        ).astype(bf16)                                 # [4, P, 8, 512]
        x8t = np.ascontiguousarray(
            xr[:, :, :NACT].reshape(8, P, 4, TQ).transpose(2, 1, 0, 3)
        ).astype(fp8)                                  # [4, P, 8, 448]
        in_maps.append(
            {
                "x16t": x16t,
                "x8t": x8t,
                "wqk8": wqk8,
                "wvi": wvi,
                "ebt": eb_pack,
            }
        )

    from concourse import bass2jax

    try:
        results = bass2jax.run_bass_via_pjrt(nc, in_maps, n_cores=NCORES)
    except Exception:
        import time as _time

        _time.sleep(2.0)
        results = bass2jax.run_bass_via_pjrt(nc, in_maps, n_cores=NCORES)

    out = np.empty((B, T, H), dtype=np.float32)
    inv = np.argsort(full_perm)
    for b in range(NCORES):
        outt = np.asarray(results[b]["outt"]).astype(np.float32)  # [NJ, H, TQ]
        outr = np.asarray(results[b]["outr"]).astype(np.float32)  # [1, NJ, TQ]
        outt /= outr.reshape(NJ, 1, TQ)
        oattn = outt.transpose(0, 2, 1).reshape(NACT, H)
        outv = np.asarray(results[b]["outv"]).astype(np.float32)  # [H, NTAIL]
        full = np.concatenate([oattn, outv.T], axis=0)  # [T, H] in perm order
        out[b] = full[inv]
    return out


# revision 20
# speedup vs baseline: 1.1789x; 1.0190x over previous
"""Trainium2 Bass kernel for nn_ExactSpectralHead (sparse resonance attention).

Reference (per batch b):  q,k,v = x@W{q,k,v}.T;  s = qk^T/sqrt(C) + bias;
  p = softmax(where(allowed, s, -inf));  out = p@v.

Strategy (8 cores, one batch element per core):
  - Positions whose `allowed` row AND column are diagonal-only ("empty"
    positions: no shared basis-prime factor with anyone) attend only to
    themselves, and nobody attends to them => out = v for those rows.
    Permute the sequence so the 1730 non-empty positions (plus 62 empty
    pads) occupy slots 0..1791; the trailing 256 empty rows are served by
    a direct v copy. The attention problem shrinks to 1792x1792.
  - EB = exp(bias)*allowed folded host-side (bf16, values are small ints,
    exact). p_raw = exp(qk/sqrt(C)) * EB; normalization deferred to a
    row-sum after the PV matmul.
  - Tiles: key chunks of 128 (PSUM partitions) x query blocks of 448.
    Per (chunk, block) tile the host computes the active column range
    from EB; fully-zero tiles are skipped; chunk pairs share a union
    range so exp/mul/matmuls all run narrowed.
  - Q/K projections in fp8 with DoubleRow perf mode (2 contraction rows
    per partition -> 2x rate; q/k only feed the exponent, accuracy ok).
    V stays bf16, computed as VT = Wv.T^T @ xT then PE-transposed into
    [tk,H] chunks for the PV stationary operand.
  - All post-softmax elementwise work is 2-byte dtype in SBUF so the DVE
    runs in its 4x/2x fast modes: exp (ACT) -> ptb fp16, EB-mul (DVE 4x)
    -> pt fp16, chunk-pair adds (DVE 4x) -> pair sums; rowsum = small
    bf16 ones-matmuls over pair sums; 1/rowsum via reciprocal_approx_fast.
  - PE stream kept dense (pstate ramp) by interleaving projection and
    transpose matmuls between score matmuls; OT(PV) matmuls trail the
    score matmuls by >=2 pairs so the ACT/DVE chase never stalls the PE.
"""

import sys

sys.path.insert(0, "/opt/trn_rl_repo")

import numpy as np
import ml_dtypes

import concourse.bass as bass
import concourse.tile as tile
import concourse.mybir as mybir

# ----------------------------------------------------------------------------
# Workaround for walrus codegen "Too many sync wait commands" on the
# TileContext tail Drain: split the global-clock sem waits across multiple SP
# NOP instructions instead of attaching them all to the single Drain.
from concourse.vector_clock import ScopedClock, VectorClock


def _split_drain_and_barrier(self, tick_clock, wait_clock):
    import concourse.mybir as _mybir

    nc = self.nc
    gc = tick_clock.global_clock
    n = len(gc)
    for p in range(n):
        t = gc[p]
        if t > 0:
            nop = nc.sync.nop(nofuse=True, hint=f"drain_wait_{p}")
            vc = VectorClock([t if i == p else 0 for i in range(n)])
            wait_clock.add_sem_waits(nop.ins, ScopedClock({None: vc}))

    tail_sem = nc.alloc_semaphore("tile_tail_sem")
    n_signals = 0
    for etype, eng in nc.engines.items():
        if etype == _mybir.EngineType.Pool:
            continue
        eng.drain(fusable=False)
        eng.sem_inc(tail_sem, 1)
        n_signals += 1
    nc.gpsimd.wait_ge(tail_sem, n_signals)
    assert self.sems is not None
    popped = nc._tile_sem_poison_stack.pop()
    assert popped is self._sem_poison
    nc.clear_and_free_semaphores(list(self.sems.allocated().values()))
    nc.gpsimd.sem_clear(range(tail_sem.num, tail_sem.num + 1))


tile.TileContext._drain_and_barrier = _split_drain_and_barrier
# ----------------------------------------------------------------------------


def _split_excess_waits(nc, max_waits=1):
    """Walrus codegen supports only one sem-wait per instruction; hoist excess
    waits onto preceding same-engine NOPs, and replace the slow EventSemaphore
    ops with NoOps carrying the same sync_info."""
    for f in nc.m.functions:
        for bb in f.blocks:
            new = []
            changed = False
            for inst in bb.instructions:
                if isinstance(inst, mybir.InstEventSemaphore):
                    si = inst.sync_info
                    changed = True
                    w = list(si.on_wait) if si else []
                    u = list(si.on_update) if si else []
                    if w:
                        new.append(
                            mybir.InstNoOp(
                                name=f"{inst.name}-wait",
                                engine=inst.engine,
                                bass_nofuse=True,
                                sync_info=mybir.SyncInfo(on_wait=w, on_update=[]),
                            )
                        )
                    new.append(
                        mybir.InstNoOp(
                            name=inst.name,
                            engine=inst.engine,
                            bass_nofuse=True,
                            sync_info=mybir.SyncInfo(on_wait=[], on_update=u),
                        )
                    )
                    continue
                si = inst.sync_info
                waits = list(si.on_wait) if si is not None else []
                if len(waits) > max_waits:
                    changed = True
                    excess, keep = waits[:-max_waits], waits[-max_waits:]
                    for k, w in enumerate(excess):
                        new.append(
                            mybir.InstNoOp(
                                name=f"{inst.name}-w{k}",
                                engine=inst.engine,
                                bass_nofuse=True,
                                sync_info=mybir.SyncInfo(on_wait=[w], on_update=[]),
                            )
                        )
                    inst.sync_info = mybir.SyncInfo(
                        on_wait=keep, on_update=list(si.on_update)
                    )
                new.append(inst)
            if changed:
                bb.instructions = new


B, T, C, H = 8, 2048, 1024, 128
NCORES = 8
SCALE = float(C) ** -0.5
P = 128
TQ = 448                   # query block width
NJ = 4                     # query blocks (4*448 = 1792 active positions)
NACT = NJ * TQ             # 1792
NKC = NACT // P            # 14 key chunks
NTAIL = T - NACT           # 256 empty-tail positions served by v-copy
BF16 = mybir.dt.bfloat16
FP16 = mybir.dt.float16
FP8 = mybir.dt.float8e4
F32 = mybir.dt.float32
DR = mybir.MatmulPerfMode.DoubleRow

_nc_cache = None
_sched_cache = None


def _schedule(allowed):
    """Permutation + per-block pair schedule, derived from `allowed`."""
    allowed = np.asarray(allowed, dtype=bool)
    row1 = allowed.sum(1) == 1
    col1 = allowed.sum(0) == 1
    empty = row1 & col1
    nonempty_idx = np.where(~empty)[0]
    empty_idx = np.where(empty)[0]
    npad = NACT - len(nonempty_idx)
    assert npad >= 0
    perm = np.concatenate([nonempty_idx, empty_idx[:npad]])
    full_perm = np.concatenate([perm, empty_idx[npad:]])
    nz = allowed[np.ix_(perm, perm)]  # [q, k]

    blocks = []  # per block: list of (i0, i1|None, lo, hi)
    for j in range(NJ):
        bq = nz[j * TQ:(j + 1) * TQ]
        act = []
        for i in range(NKC):
            colnz = bq[:, i * P:(i + 1) * P].any(axis=1)
            if not colnz.any():
                continue
            qlo = int(np.argmax(colnz))
            qhi = TQ - int(np.argmax(colnz[::-1]))
            act.append((i, qlo, qhi))
        idxs = [a[0] for a in act]
        assert idxs == list(range(len(idxs))), f"non-contiguous actives {idxs}"
        pairs = []
        k = 0
        while k < len(act):
            if k + 1 < len(act):
                lo = min(act[k][1], act[k + 1][1])
                hi = max(act[k][2], act[k + 1][2])
                pairs.append([act[k][0], act[k + 1][0], lo, hi])
                k += 2
            else:
                pairs.append([act[k][0], None, act[k][1], act[k][2]])
                k += 1
        pairs[0][2], pairs[0][3] = 0, TQ  # first pair covers full width
        blocks.append([tuple(p) for p in pairs])

    # eb packing offsets (elements per partition), consumption order
    offs = []
    off = 0
    for j in range(NJ):
        boffs = []
        for (i0, i1, lo, hi) in blocks[j]:
            w = hi - lo
            n = 2 if i1 is not None else 1
            boffs.append(off)
            off += n * w
        offs.append(boffs)
    return full_perm, blocks, offs, off


def _build_nc(blocks, offs, ebw):
    nc = bass.Bass()
    # host-packed DRAM inputs
    x16t = nc.declare_dram_parameter("x16t", [4, P, 8, 512], BF16, isOutput=False)
    x8t = nc.declare_dram_parameter("x8t", [4, P, 8, TQ], FP8, isOutput=False)
    wqk8 = nc.declare_dram_parameter("wqk8", [P, 2, 4, 2, H], FP8, isOutput=False)
    # wv chunks + identity packed in one bf16 buffer
    wvi = nc.declare_dram_parameter("wvi", [P, 8 * H + P], BF16, isOutput=False)
    ebt = nc.declare_dram_parameter("ebt", [P, ebw], BF16, isOutput=False)
    outt = nc.declare_dram_parameter("outt", [NJ, H, TQ], FP16, isOutput=True)
    outr = nc.declare_dram_parameter("outr", [1, NJ, TQ], FP16, isOutput=True)
    outv = nc.declare_dram_parameter("outv", [P, NTAIL], BF16, isOutput=True)

    with tile.TileContext(nc) as tc:
        with (
            tc.tile_pool(name="const", bufs=1) as const,
            tc.tile_pool(name="qkv_psum", bufs=1, space="PSUM") as qkv_psum,
            tc.tile_pool(name="st_psum", bufs=2, space="PSUM") as st_psum,
            tc.tile_pool(name="ot_psum", bufs=2, space="PSUM") as ot_psum,
            tc.tile_pool(name="rs_psum", bufs=1, space="PSUM") as rs_psum,
            tc.tile_pool(name="ptb", bufs=3) as ptb_pool,
            tc.tile_pool(name="pt", bufs=8) as pt_pool,
            tc.tile_pool(name="ps", bufs=9) as ps_pool,
            tc.tile_pool(name="outs", bufs=2) as out_pool,
        ):
            x_sb = const.tile([P, 4, 8, 512], BF16, tag="x", name="x_sb")
            x8_sb = const.tile([P, 4, 8, TQ], FP8, tag="x8", name="x8_sb")
            w8_sb = const.tile([P, 2, 4, 2, H], FP8, tag="w8", name="w8_sb")
            wvi_sb = const.tile([P, 8 * H + P], BF16, tag="wvi", name="wvi_sb")
            rsn_sb = const.tile([1, NJ, TQ], FP16, tag="rsn", name="rsn_sb")
            eb_sb = const.tile([P, ebw], BF16, tag="eb", name="eb_sb")
            QT_sb = const.tile([P, NACT], BF16, tag="QT", name="QT_sb")
            KT_sb = const.tile([P, NACT], BF16, tag="KT", name="KT_sb")
            VT_sb = const.tile([P, T], BF16, tag="VT", name="VT_sb")
            v_sb = const.tile([P, NKC, H], BF16, tag="v", name="v_sb")
            ones_sb = const.tile([P, P], BF16, tag="ones", name="ones_sb")
            nc.vector.memset(ones_sb[:], 1.0)
            wv_sb = wvi_sb[:, 0:8 * H].rearrange("p (c h) -> p c h", c=8)
            id_sb = wvi_sb[:, 8 * H:8 * H + P]

            # ---- t0 DMA batch: critical-path pieces first on hardware queues
            with tc.high_priority():
                nc.scalar.dma_start(w8_sb[:], wqk8[:])
                nc.sync.dma_start(x8_sb[:, 0], x8t[0])
                nc.sync.dma_start(x8_sb[:, 1], x8t[1])
                nc.sync.dma_start(eb_sb[:, offs[0][0]:offs[1][0]],
                                  ebt[:, offs[0][0]:offs[1][0]])
                nc.sync.dma_start(x8_sb[:, 2], x8t[2])
                nc.sync.dma_start(x8_sb[:, 3], x8t[3])
                nc.gpsimd.dma_start(x_sb[:, 0, 0:4], x16t[0][:, 0:4])
                nc.gpsimd.dma_start(x_sb[:, 0, 4:8], x16t[0][:, 4:8])
                nc.gpsimd.dma_start(wvi_sb[:], wvi[:])
                nc.gpsimd.dma_start(x_sb[:, 1], x16t[1])
                nc.gpsimd.dma_start(eb_sb[:, offs[1][0]:offs[2][0]],
                                    ebt[:, offs[1][0]:offs[2][0]])
                nc.gpsimd.dma_start(x_sb[:, 2], x16t[2])
                nc.gpsimd.dma_start(eb_sb[:, offs[2][0]:offs[3][0]],
                                    ebt[:, offs[2][0]:offs[3][0]])
                nc.gpsimd.dma_start(x_sb[:, 3], x16t[3])
                nc.gpsimd.dma_start(eb_sb[:, offs[3][0]:ebw], ebt[:, offs[3][0]:ebw])

            # ---------- emission helpers ----------
            def qk(j):
                """Q/K projections for block j (fp8 DoubleRow, 4 c-pairs)."""
                for wsel, dst in ((0, QT_sb), (1, KT_sb)):
                    ps = qkv_psum.tile([P, 512], F32, tag="qkvps", name="qkvps")
                    for pair in range(4):
                        nc.tensor.matmul(
                            ps[:, :TQ],
                            lhsT=w8_sb[:, wsel, pair],
                            rhs=x8_sb[:, j, 2 * pair:2 * pair + 2, :],
                            start=(pair == 0),
                            stop=(pair == 3),
                            perf_mode=DR,
                        )
                    nc.vector.tensor_copy(dst[:, j * TQ:(j + 1) * TQ], ps[:, :TQ])

            def vt(vb):
                """V^T projection for 512-col block vb (bf16)."""
                ps = qkv_psum.tile([P, 512], F32, tag="qkvps", name="qkvps")
                for c in range(8):
                    nc.tensor.matmul(
                        ps[:],
                        lhsT=wv_sb[:, c, :],
                        rhs=x_sb[:, vb, c, :],
                        start=(c == 0),
                        stop=(c == 7),
                    )
                nc.vector.tensor_copy(VT_sb[:, vb * 512:(vb + 1) * 512], ps[:])
                if vb == 3:
                    nc.sync.dma_start(outv[:], VT_sb[:, NACT:T])

            def tr(m):
                """PE-transpose VT chunk m into v_sb[:, m, :]."""
                pst = qkv_psum.tile([P, 1024], BF16, tag="qkvps", name="qkvps")
                nc.tensor.transpose(pst[:, :P], VT_sb[:, m * P:(m + 1) * P], id_sb)
                nc.vector.tensor_copy(v_sb[:, m, :], pst[:, :P])

            class Blk:
                pass

            def blk_start(j):
                b = Blk()
                b.j = j
                b.pairs = blocks[j]
                b.ot = ot_psum.tile([P, 512], F32, tag="ot", name="ot")
                b.rs = rs_psum.tile([P, 512], F32, tag="rs", name="rs")
                b.pts = []
                b.qsums = []
                b.nt = sum(2 if i1 is not None else 1 for (i0, i1, _, _) in b.pairs)
                b.ti = 0
                return b

            def st(b, p):
                """Score matmuls + exp + EB-mul + pair-sum for pair p."""
                i0, i1, lo, hi = b.pairs[p]
                n = 2 if i1 is not None else 1
                st2 = st_psum.tile([P, 2, 512], F32, tag="st", name="st2")
                for k, i in enumerate((i0, i1)[:n]):
                    nc.tensor.matmul(
                        st2[:, k, lo:hi],
                        lhsT=KT_sb[:, i * P:(i + 1) * P],
                        rhs=QT_sb[:, b.j * TQ + lo:b.j * TQ + hi],
                        start=True,
                        stop=True,
                    )
                ptb = ptb_pool.tile([P, 2, TQ], FP16, tag="ptb", name="ptb")
                nc.scalar.activation(
                    ptb[:, :n, lo:hi], st2[:, :n, lo:hi],
                    mybir.ActivationFunctionType.Exp, scale=SCALE,
                )
                pt = pt_pool.tile([P, 2, TQ], FP16, tag="pt", name="pt")
                off = offs[b.j][p]
                w = hi - lo
                for k in range(n):
                    nc.vector.tensor_mul(
                        pt[:, k, lo:hi],
                        ptb[:, k, lo:hi],
                        eb_sb[:, off + k * w:off + (k + 1) * w],
                    )
                b.pts.append(pt)
                if p % 2 == 0:
                    # new quad accumulator (pair p's range contains pair p+1's)
                    qs = ps_pool.tile([P, TQ], FP16, tag="psum", name="psum")
                    if n == 2:
                        nc.vector.tensor_add(qs[:, lo:hi], pt[:, 0, lo:hi], pt[:, 1, lo:hi])
                    else:
                        nc.vector.tensor_copy(qs[:, lo:hi], pt[:, 0, lo:hi])
                    b.qsums.append((qs, lo, hi))
                else:
                    qs, qlo, qhi = b.qsums[-1]
                    assert qlo <= lo and hi <= qhi, "pair ranges not nested"
                    for k in range(n):
                        nc.vector.tensor_add(qs[:, lo:hi], qs[:, lo:hi], pt[:, k, lo:hi])

            def ot(b, p):
                i0, i1, lo, hi = b.pairs[p]
                n = 2 if i1 is not None else 1
                for k, i in enumerate((i0, i1)[:n]):
                    nc.tensor.matmul(
                        b.ot[:, lo:hi],
                        lhsT=v_sb[:, i, :],
                        rhs=b.pts[p][:, k, lo:hi],
                        start=(b.ti == 0),
                        stop=(b.ti == b.nt - 1),
                        skip_group_check=True,
                    )
                    b.ti += 1

            def rs_all(b):
                nq = len(b.qsums)
                for q, (qs, lo, hi) in enumerate(b.qsums):
                    nc.tensor.matmul(
                        b.rs[:, lo:hi],
                        lhsT=ones_sb[:],
                        rhs=qs[:, lo:hi],
                        start=(q == 0),
                        stop=(q == nq - 1),
                        skip_group_check=True,
                    )

            def epi(b):
                otn = out_pool.tile([P, TQ], FP16, tag="otn", name="otn")
                nc.vector.tensor_copy(otn[:], b.ot[:, :TQ])
                nc.vector.tensor_copy(rsn_sb[:, b.j, :], b.rs[0:1, :TQ])
                nc.sync.dma_start(outt[b.j], otn[:])
                if b.j == NJ - 1:
                    nc.sync.dma_start(outr[:], rsn_sb[:])

            # ---------- global emission ----------
            # OT/RS for block j-1 are interleaved into block j's ST phase so
            # the PE always has ready work while the exp/mul chase runs.
            qk(0)
            qk(1)
            qk(2)

            bs = [blk_start(j) for j in range(NJ)]

            def fillers_for(j):
                if j == 0:
                    return [lambda: vt(0),
                            lambda: tr(0), lambda: tr(1), lambda: tr(2),
                            lambda: tr(3), lambda: qk(3), lambda: vt(1)]
                if j == 1:
                    return [lambda: tr(4), lambda: tr(5), lambda: tr(6),
                            lambda: tr(7), lambda: vt(2)]
                if j == 2:
                    return [lambda: tr(8), lambda: tr(9), lambda: tr(10),
                            lambda: tr(11), lambda: vt(3)]
                return [lambda: tr(12), lambda: tr(13)]

            for j in range(NJ):
                b = bs[j]
                prev = bs[j - 1] if j > 0 else None
                fill = fillers_for(j)
                lag = list(range(len(prev.pairs))) if prev is not None else []
                li = 0
                fi = 0
                own = 0
                for p in range(len(b.pairs)):
                    # pad with one lagged OT pair + one filler per ST pair
                    if li < len(lag):
                        ot(prev, lag[li])
                        li += 1
                    if fi < len(fill):
                        fill[fi]()
                        fi += 1
                    st(b, p)
                    if j == NJ - 1 and p >= 4:
                        # last block: start its own PV early to shrink the tail
                        if li < len(lag):
                            ot(prev, li)
                            li += 1
                        ot(b, own)
                        own += 1
                while fi < len(fill):
                    fill[fi]()
                    fi += 1
                while li < len(lag):
                    ot(prev, li)
                    li += 1
                if prev is not None:
                    rs_all(prev)
                    epi(prev)
                if j == NJ - 1:
                    while own < len(b.pairs):
                        ot(b, own)
                        own += 1
                    rs_all(b)
                    epi(b)

    _split_excess_waits(nc)
    return nc


def kernel(x, Wq, Wk, Wv, resonance_bias, allowed):
    global _nc_cache, _sched_cache
    x = np.asarray(x, dtype=np.float32)
    Wq = np.asarray(Wq, dtype=np.float32)
    Wk = np.asarray(Wk, dtype=np.float32)
    Wv = np.asarray(Wv, dtype=np.float32)
    resonance_bias = np.asarray(resonance_bias, dtype=np.float32)
    allowed = np.asarray(allowed)

    bf16 = ml_dtypes.bfloat16
    fp8 = ml_dtypes.float8_e4m3

    if _sched_cache is None:
        _sched_cache = _schedule(allowed)
    full_perm, blocks, offs, ebw = _sched_cache
    if _nc_cache is None:
        _nc_cache = _build_nc(blocks, offs, ebw)
    nc = _nc_cache

    # ---- host packing ----
    EB = np.exp(resonance_bias) * allowed
    EBp = EB[np.ix_(full_perm[:NACT], full_perm[:NACT])]  # [q, k]
    ebT = np.ascontiguousarray(EBp.T)                      # [k, q]
    eb_pack = np.empty((P, ebw), dtype=bf16)
    for j in range(NJ):
        for p, (i0, i1, lo, hi) in enumerate(blocks[j]):
            off = offs[j][p]
            w = hi - lo
            qs = slice(j * TQ + lo, j * TQ + hi)
            eb_pack[:, off:off + w] = ebT[i0 * P:(i0 + 1) * P, qs].astype(bf16)
            if i1 is not None:
                eb_pack[:, off + w:off + 2 * w] = ebT[i1 * P:(i1 + 1) * P, qs].astype(bf16)

    wqk8 = np.ascontiguousarray(
        np.stack(
            [w.reshape(4, 2, P, H).transpose(2, 0, 1, 3)
             for w in (np.ascontiguousarray(Wq.T), np.ascontiguousarray(Wk.T))],
            axis=1,
        ).astype(fp8)
    )
    wvi = np.concatenate(
        [
            Wv.T.reshape(8, P, H).transpose(1, 0, 2).reshape(P, 8 * H),
            np.eye(P, dtype=np.float32),
        ],
        axis=1,
    ).astype(bf16)

    in_maps = []
    for b in range(NCORES):
        xT = x[b].T[:, full_perm]                      # [C, T] permuted cols
        xr = xT.reshape(8, P, T)                       # [c, p, t]
        x16t = np.ascontiguousarray(
            xr.reshape(8, P, 4, 512).transpose(2, 1, 0, 3)
        ).astype(bf16)                                 # [4, P, 8, 512]
        x8t = np.ascontiguousarray(
            xr[:, :, :NACT].reshape(8, P, 4, TQ).transpose(2, 1, 0, 3)
        ).astype(fp8)                                  # [4, P, 8, 448]
        in_maps.append(
            {
                "x16t": x16t,
                "x8t": x8t,
                "wqk8": wqk8,
                "wvi": wvi,
                "ebt": eb_pack,
            }
        )

    from concourse import bass2jax

    try:
        results = bass2jax.run_bass_via_pjrt(nc, in_maps, n_cores=NCORES)
    except Exception:
        import time as _time

        _time.sleep(2.0)
        results = bass2jax.run_bass_via_pjrt(nc, in_maps, n_cores=NCORES)

    out = np.empty((B, T, H), dtype=np.float32)
    inv = np.argsort(full_perm)
    for b in range(NCORES):
        outt = np.asarray(results[b]["outt"]).astype(np.float32)  # [NJ, H, TQ]
        outr = np.asarray(results[b]["outr"]).astype(np.float32)  # [1, NJ, TQ]
        outt /= outr.reshape(NJ, 1, TQ)
        oattn = outt.transpose(0, 2, 1).reshape(NACT, H)
        outv = np.asarray(results[b]["outv"]).astype(np.float32)  # [H, NTAIL]
        full = np.concatenate([oattn, outv.T], axis=0)  # [T, H] in perm order
        out[b] = full[inv]
    return out
